# revision 28
# baseline (speedup 1.0000x reference)
"""Trainium2 kernel for the CLML loss function.

Math: nuclear_norm(diag(m_c) F) = tr(sqrt(G_c)) with G_c = F^T diag(m_c) F a
256x256 PSD Gram matrix.  tr(sqrt(.)) is evaluated with a matmul-only
Chebyshev trace method (degree 4):

  A  = G*s - kappa*I          (affine map of the spectrum into [-1, 1])
  T2 = 2*A*A - I
  tr(T2) = 2<A,A> - 256,  tr(T4) = 2<T2,T2> - 256,  tr(T3) = 2<T2,A> - tr(T1)

The host combines the traces with Chebyshev coefficients of sqrt(x + kappa).
tr(G_c) (hence the scale s) is computed host-side from fp32 row norms, so the
device only produces the three inner products per matrix.

Sharding: each core handles 8 classes as 4 pairs.  Pair 0's rows are sorted
into segments (11, 10, 01, 00) covering ALL N rows, so the full-matrix Gram
G_all = S11+S10+S01+S00 falls out for free.  Pairs 1-3 use the complement
trick: only segments (00, 10, 01) are contracted (~64% of rows) and
G_c0 = G_all - S00 - S01,  G_c1 = G_all - S00 - S10.

Features are fp8 e3m4 (4 mantissa bits; inputs are ~N(0,1)); the Chebyshev
recurrence runs in bf16.  Element-wise work is spread over DVE (assembly,
cross inner products), ACT (square inner products) and Pool (PSUM drains).
"""

import numpy as np
import ml_dtypes
from contextlib import ExitStack

import concourse.bass as bass
import concourse.mybir as mybir
import concourse.tile as tile
from concourse import bacc
from concourse.bass_utils import run_bass_kernel_spmd

# ---- problem constants (hardcoded; harness provides identical shapes) ----
N, C, D = 8192, 64, 256
P = 128
TAU = 0.7
MARGIN = 1.0
DELTA = 1.0

# Chebyshev spectral interval, relative to mean eigenvalue mu = tr(G)/D.
ALPHA, BETA = 0.45, 1.9
LC = (BETA + ALPHA) / 2.0
LH = (BETA - ALPHA) / 2.0
KAPPA = LC / LH
DEG = 4
IPC = 3

BF16 = mybir.dt.bfloat16
F32 = mybir.dt.float32
DT_FEAT = mybir.dt.float8e3
NP_FEAT = ml_dtypes.float8_e3m4
NP_BF16 = ml_dtypes.bfloat16

TRACE = False
LAST_RESULT = None

_PROGRAM_CACHE = {}


def _build_program(b0, a0, c0, z0, zc, cc, ac):
    """b0,a0,c0,z0: chunk counts of pair0's (11, 10, 01, 00) segments;
    zc,cc,ac: chunk counts of the complement pairs' (00, 01, 10) segments.
    Shared by all pairs and cores (zero-padded on host)."""
    CP0 = b0 + a0 + c0 + z0
    CPQ = zc + cc + ac
    CPT = CP0 + 3 * CPQ
    nc = bacc.Bacc(
        "TRN2",
        target_bir_lowering=False,
        debug=False,
        enable_asserts=False,
        num_devices=8,
    )
    fsort = nc.dram_tensor("fsort", [P, CPT * D], DT_FEAT, kind="ExternalInput").ap()
    cf32 = nc.dram_tensor("cf32", [P, 400], F32, kind="ExternalInput").ap()
    cbf16 = nc.dram_tensor("cbf16", [P, 512], BF16, kind="ExternalInput").ap()
    out_ip = nc.dram_tensor("out_ip", [P, 9 * IPC], F32, kind="ExternalOutput").ap()

    alu = mybir.AluOpType
    aft = mybir.ActivationFunctionType

    with tile.TileContext(nc) as tc, ExitStack() as ctx:
        f0pool = ctx.enter_context(tc.tile_pool(name="f0", bufs=1))
        fqpool = ctx.enter_context(tc.tile_pool(name="fq", bufs=3))
        cpool = ctx.enter_context(tc.tile_pool(name="c", bufs=1))
        gpool = ctx.enter_context(tc.tile_pool(name="gall", bufs=1))
        wpool = ctx.enter_context(tc.tile_pool(name="w", bufs=8))
        apool = ctx.enter_context(tc.tile_pool(name="amat", bufs=9))
        scrpool = ctx.enter_context(tc.tile_pool(name="scr", bufs=4))
        opool = ctx.enter_context(tc.tile_pool(name="outs", bufs=1))
        p0sum = ctx.enter_context(tc.tile_pool(name="p0", bufs=1, space="PSUM"))
        pqsum = ctx.enter_context(tc.tile_pool(name="pq", bufs=2, space="PSUM"))
        trsum = ctx.enter_context(tc.tile_pool(name="tr", bufs=1, space="PSUM"))

        # ---- input tiles + DMA (partition-major contiguous) ----
        fs0 = f0pool.tile([P, CP0, D], DT_FEAT, tag="f0")
        fsq = [fqpool.tile([P, CPQ, D], DT_FEAT, tag="fq", name=f"fq{q}")
               for q in range(3)]

        def dma_chunks(dst, base, cnt, nsplit, head=None):
            splits = [cnt * i // nsplit for i in range(nsplit + 1)]
            if head is not None:
                splits = [0] + [s for s in splits if s > head]
                splits.insert(1, head)
            for r0, r1 in zip(splits, splits[1:]):
                nc.sync.dma_start(
                    dst[:, r0:r1], fsort[:, (base + r0) * D : (base + r1) * D]
                )

        dma_chunks(fs0, 0, CP0, 9, head=2)
        cfp = cpool.tile([P, 400], F32, tag="cf")
        nc.sync.dma_start(cfp[:], cf32)
        cbt = cpool.tile([P, 512], BF16, tag="cb")
        nc.sync.dma_start(cbt[:], cbf16)
        for q in range(3):
            dma_chunks(fsq[q], CP0 + q * CPQ, CPQ, 6)

        kI = cfp[:, 0:384]        # kappa at [p, p] (top) and [p, 256+p] (br)
        svec = cfp[:, 384:400]    # per-class scale s_j at col j (j=0..8)
        T0 = cbt[:, 0:512]        # identity in [128, 512] two-row-block layout

        ip_sb = opool.tile([P, 9 * IPC], F32, tag="ip")

        gall = gpool.tile([P, 384], F32, tag="g")

        def asm_stt(j, src):
            """A_j = s_j * src - kappa*I; src is f32 [P, 384] (top+br)."""
            s = svec[:, j : j + 1]
            A = apool.tile([P, 512], BF16, tag="a", name=f"A{j}")
            nc.vector.scalar_tensor_tensor(
                A[:, 0:256], src[:, 0:256], s, kI[:, 0:256], alu.mult, alu.subtract
            )
            nc.vector.scalar_tensor_tensor(
                A[:, 384:512], src[:, 256:384], s, kI[:, 256:384],
                alu.mult, alu.subtract,
            )
            return A

        def asm_tr(A):
            # A10 = A01^T into [256:384] so A[:, 256:512] is the bottom rows
            ptr = trsum.tile([P, 128], BF16, tag="t")
            nc.tensor.transpose(ptr[:], A[:, 128:256], T0[:, 0:128])
            nc.vector.tensor_copy(A[:, 256:384], ptr[:])

        def asm_A(j, src):
            A = asm_stt(j, src)
            asm_tr(A)
            return A

        def gram_pair0():
            pg = p0sum.tile([P, 1536], F32, tag="g0")
            tops = [pg[:, i * 256 : (i + 1) * 256] for i in range(4)]
            brs = [pg[:, 1024 + i * 128 : 1024 + (i + 1) * 128] for i in range(4)]
            bounds = [0, b0, b0 + a0, b0 + a0 + c0, CP0]
            for i in range(4):
                lo, hi = bounds[i], bounds[i + 1]
                for n in range(lo, hi):
                    Fn = fs0[:, n]
                    nc.tensor.matmul(
                        tops[i], Fn[:, 0:128], Fn, start=(n == lo), stop=(n == hi - 1)
                    )
                    nc.tensor.matmul(
                        brs[i], Fn[:, 128:256], Fn[:, 128:256],
                        start=(n == lo), stop=(n == hi - 1),
                    )
            return pg, tops, brs

        def drain_pair0_stt(tops, brs):
            # GPSIMD has no PSUM access and engines take at most one PSUM
            # operand per op: ACT copies S11/S01 out, DVE adds with one PSUM
            # side.
            c11 = wpool.tile([P, 384], F32, tag="w", name="c11")
            c01 = wpool.tile([P, 384], F32, tag="w", name="c01")
            t01 = wpool.tile([P, 384], F32, tag="w", name="t01")
            t23 = wpool.tile([P, 384], F32, tag="w", name="t23")
            t02 = wpool.tile([P, 384], F32, tag="w", name="t02")
            nc.scalar.copy(c11[:, 0:256], tops[0])
            nc.scalar.copy(c11[:, 256:384], brs[0])
            nc.scalar.copy(c01[:, 0:256], tops[2])
            nc.scalar.copy(c01[:, 256:384], brs[2])
            nc.vector.tensor_add(t01[:, 0:256], c11[:, 0:256], tops[1])
            nc.vector.tensor_add(t01[:, 256:384], c11[:, 256:384], brs[1])
            nc.vector.tensor_add(t23[:, 0:256], c01[:, 0:256], tops[3])
            nc.vector.tensor_add(t23[:, 256:384], c01[:, 256:384], brs[3])
            nc.vector.tensor_add(t02[:], c11[:], c01[:])
            nc.vector.tensor_add(gall[:], t01[:], t23[:])
            return asm_stt(0, t01), asm_stt(1, t02), asm_stt(8, gall)

        def gram_pairq(q, fillers=()):
            # segments: 0 -> 00, 1 -> 01, 2 -> 10.  The 00 segment's br
            # matmuls accumulate into BOTH classes' br accumulators directly
            # (br0 = S00b+S01b, br1 = S00b+S10b) to fit the pair in 2 banks.
            # `fillers`: (after_chunk, fn) callbacks emitted mid-stream so
            # trailing cheb matmuls are spaced out in the PE queue.
            fst = fsq[q - 1]
            pg = pqsum.tile([P, 1024], F32, tag="gq", name=f"gq{q}")
            tops = [pg[:, i * 256 : (i + 1) * 256] for i in range(3)]
            br0 = pg[:, 768:896]
            br1 = pg[:, 896:1024]
            bounds = [0, zc, zc + cc, CPQ]
            fill = sorted(fillers, key=lambda x: x[0], reverse=True)
            for i in range(3):
                lo, hi = bounds[i], bounds[i + 1]
                for n in range(lo, hi):
                    Fn = fst[:, n]
                    nc.tensor.matmul(
                        tops[i], Fn[:, 0:128], Fn, start=(n == lo), stop=(n == hi - 1)
                    )
                    Fb = Fn[:, 128:256]
                    if i == 0:
                        nc.tensor.matmul(br0, Fb, Fb, start=(n == lo), stop=False)
                        nc.tensor.matmul(br1, Fb, Fb, start=(n == lo), stop=False)
                    elif i == 1:
                        nc.tensor.matmul(br0, Fb, Fb, start=False, stop=(n == hi - 1))
                    else:
                        nc.tensor.matmul(br1, Fb, Fb, start=False, stop=(n == hi - 1))
                    while fill and fill[-1][0] <= n:
                        fill.pop()[1](tops, (br0, br1))
            while fill:
                fill.pop()[1](tops, (br0, br1))
            return pg, tops, (br0, br1)

        def drain_classA(q, tops, brs):
            """class 2q: G = gall - (S00 + S01); ready once segs 00,01 stop."""
            br0, br1 = brs
            c00 = wpool.tile([P, 256], F32, tag="w", name=f"c00_{q}")
            u0 = wpool.tile([P, 256], F32, tag="w", name=f"u0_{q}")
            w0 = wpool.tile([P, 384], F32, tag="w", name=f"w0_{q}")
            nc.scalar.copy(c00[:], tops[0])
            nc.vector.tensor_add(u0[:], c00[:], tops[1])
            nc.vector.tensor_sub(w0[:, 0:256], gall[:, 0:256], u0[:])
            nc.vector.tensor_sub(w0[:, 256:384], gall[:, 256:384], br0)
            return asm_stt(2 * q, w0), c00

        def drain_classB(q, tops, brs, c00):
            """class 2q+1: G = gall - (S00 + S10)."""
            br0, br1 = brs
            u1 = wpool.tile([P, 256], F32, tag="w", name=f"u1_{q}")
            w1 = wpool.tile([P, 384], F32, tag="w", name=f"w1_{q}")
            nc.vector.tensor_add(u1[:], c00[:], tops[2])
            nc.vector.tensor_sub(w1[:, 0:256], gall[:, 0:256], u1[:])
            nc.vector.tensor_sub(w1[:, 256:384], gall[:, 256:384], br1)
            return asm_stt(2 * q + 1, w1)

        def drain_pairq(q, pg, tops, brs):
            A0, c00 = drain_classA(q, tops, brs)
            asm_tr(A0)
            A1 = drain_classB(q, tops, brs, c00)
            asm_tr(A1)
            return [(A0, 2 * q), (A1, 2 * q + 1)]

        def cheb(A, j):
            """Inner products <A,A>, <P,P>, <P,A> with P = A*A left in PSUM;
            the host folds T2 = 2P - I into the trace formulas."""
            base = j * IPC
            scr = scrpool.tile([P, 512], BF16, tag="scr")
            nc.scalar.activation(
                scr[:], A[:], aft.Square, accum_out=ip_sb[:, base : base + 1]
            )
            # rotate pair0's psum buffer (drained by then); plain pool-tile
            # rotation gives a clean WAR edge vs the previous cheb's reads
            ppt = p0sum.tile([P, 1536], F32, tag="g0", name=f"pp{j}")
            pp = ppt[:, 0:512]
            for mb in (0, 1):
                pm = pp[:, mb * 256 : mb * 256 + 256]
                nc.tensor.matmul(
                    pm, A[:, mb * 128 : mb * 128 + 128], A[:, 0:256],
                    start=True, stop=False,
                )
                nc.tensor.matmul(
                    pm, A[:, 256 + mb * 128 : 256 + mb * 128 + 128], A[:, 256:512],
                    start=False, stop=True,
                )
            scr2 = scrpool.tile([P, 512], BF16, tag="scr")
            nc.scalar.activation(
                scr2[:], pp, aft.Square, accum_out=ip_sb[:, base + 1 : base + 2]
            )
            scr3 = scrpool.tile([P, 512], BF16, tag="scr")
            nc.vector.scalar_tensor_tensor(
                scr3[:], pp, 1.0, A[:], alu.mult, alu.mult,
                accum_out=ip_sb[:, base + 2 : base + 3],
            )

        # ---- schedule: pair grams lead the PE queue; each pair's drain and
        # chebs are interleaved as fillers into the NEXT pair's gram chunks,
        # so the element engines run one pair behind and psum-rotation WARs
        # never stall the PE.  The solo cheb (ready early) covers the final
        # drain's latency ----
        st = {}
        f = lambda fn: (lambda tops, brs: fn())

        def mk_drainA(q):
            def go(tops, brs):
                st[f"A{2 * q}"], st[f"c{q}"] = drain_classA(q, tops, brs)
            return go

        pg0 = gram_pair0()
        pq1 = gram_pairq(1, fillers=[
            (1, lambda tops, brs: st.update(
                zip(("A0", "A1", "A8"), drain_pair0_stt(pg0[1], pg0[2])))),
            (9, f(lambda: (asm_tr(st["A0"]), asm_tr(st["A1"]),
                           asm_tr(st["A8"])))),
            (13, f(lambda: cheb(st["A0"], 0))),
            (zc + cc - 1, mk_drainA(1)),
            (zc + cc + 1, f(lambda: cheb(st["A1"], 1))),
            (min(zc + cc + 7, CPQ - 2), f(lambda: asm_tr(st["A2"]))),
        ])
        st["A3"] = drain_classB(1, pq1[1], pq1[2], st["c1"])
        pq2 = gram_pairq(2, fillers=[
            (2, f(lambda: asm_tr(st["A3"]))),
            (6, f(lambda: cheb(st["A2"], 2))),
            (12, f(lambda: cheb(st["A3"], 3))),
            (zc + cc - 1, mk_drainA(2)),
            (min(zc + cc + 6, CPQ - 4), f(lambda: asm_tr(st["A4"]))),
            (min(zc + cc + 10, CPQ - 1), f(lambda: cheb(st["A4"], 4))),
        ])
        st["A5"] = drain_classB(2, pq2[1], pq2[2], st["c2"])
        pq3 = gram_pairq(3, fillers=[
            (2, f(lambda: asm_tr(st["A5"]))),
            (7, f(lambda: cheb(st["A5"], 5))),
            (zc + cc - 1, mk_drainA(3)),
            (min(zc + cc + 6, CPQ - 4), f(lambda: asm_tr(st["A6"]))),
            (min(zc + cc + 10, CPQ - 1), f(lambda: cheb(st["A6"], 6))),
        ])
        A7 = drain_classB(3, pq3[1], pq3[2], st["c3"])
        cheb(st["A8"], 8)  # solo matrix, ready early: covers drain latency
        asm_tr(A7)
        cheb(A7, 7)

        nc.sync.dma_start(out_ip, ip_sb[:])

    nc.compile()
    return nc


def _get_program(key):
    if key not in _PROGRAM_CACHE:
        _PROGRAM_CACHE[key] = _build_program(*key)
    return _PROGRAM_CACHE[key]


def _host_consts():
    kI = np.zeros((P, 384), np.float32)
    for p in range(P):
        kI[p, p] = KAPPA
        kI[p, 256 + p] = KAPPA
    T0 = np.zeros((P, 512), np.float32)
    for p in range(P):
        T0[p, p] = 1.0
        T0[p, 384 + p] = 1.0
    return kI, T0.astype(NP_BF16)


def kernel(logits, targets, feature, lam, epoch):
    global LAST_RESULT
    logits = np.asarray(logits, dtype=np.float32)
    targets_b = np.asarray(targets) == 1
    feature = np.asarray(feature, dtype=np.float32)
    lam_f = float(np.asarray(lam))
    relabel = int(np.asarray(epoch)) >= 1

    # masks (same fp32 semantics as the reference)
    if relabel:
        shifted = (logits - targets_b.astype(np.float32)).astype(np.float32)
        thresh = np.float32(np.log(TAU / (1.0 - TAU)))
        mask = targets_b | (shifted > thresh)
    else:
        mask = targets_b.copy()

    feat8 = np.ascontiguousarray(feature.astype(NP_FEAT))
    kI, T0 = _host_consts()

    # host-side traces: tr(G_c) = sum of masked row norms (fp64-exact)
    rn = (feature.astype(np.float64) ** 2).sum(axis=1)
    t1 = rn @ mask  # [C]
    t1_all = float(rn.sum())

    # ---- per-core, per-pair sorted row layout ----
    # pair 0: segments (11, 10, 01, 00); pairs 1-3: complement (00, 10, 01)
    idx = {}
    for k in range(8):
        m0 = mask[:, 8 * k]
        m1 = mask[:, 8 * k + 1]
        idx[(k, 0)] = [
            np.where(m0 & m1)[0], np.where(m0 & ~m1)[0],
            np.where(~m0 & m1)[0], np.where(~m0 & ~m1)[0],
        ]
        for q in range(1, 4):
            m0 = mask[:, 8 * k + 2 * q]
            m1 = mask[:, 8 * k + 2 * q + 1]
            idx[(k, q)] = [
                np.where(~m0 & ~m1)[0], np.where(~m0 & m1)[0],
                np.where(m0 & ~m1)[0],
            ]

    def nch(x):
        return max((len(x) + P - 1) // P, 1)

    cnt0 = [max(nch(idx[(k, 0)][i]) for k in range(8)) for i in range(4)]
    cntq = [max(nch(idx[(k, q)][i]) for k in range(8) for q in range(1, 4))
            for i in range(3)]
    key = tuple(cnt0) + tuple(cntq)
    CP0 = sum(cnt0)
    CPQ = sum(cntq)
    CPT = CP0 + 3 * CPQ

    in_maps = []
    for k in range(8):
        fsort = np.zeros((CPT * P, D), NP_FEAT)
        off = 0
        for q in range(4):
            cnts = cnt0 if q == 0 else cntq
            for rows, segc in zip(idx[(k, q)], cnts):
                fsort[off : off + len(rows)] = feat8[rows]
                off += segc * P
        fsort_pm = np.ascontiguousarray(
            fsort.reshape(CPT, P, D).transpose(1, 0, 2).reshape(P, CPT * D)
        )
        svec = np.zeros((P, 16), np.float32)
        for j in range(8):
            svec[:, j] = D / (LH * max(t1[8 * k + j], 1e-30))
        svec[:, 8] = D / (LH * max(t1_all, 1e-30))
        cf32 = np.ascontiguousarray(
            np.concatenate([kI, svec], axis=1).astype(np.float32)
        )
        in_maps.append({"fsort": fsort_pm, "cf32": cf32, "cbf16": T0})

    nc = _get_program(key)
    res = run_bass_kernel_spmd(nc, in_maps, core_ids=list(range(8)), trace=TRACE)
    LAST_RESULT = res

    # ---- host combination ----
    xs = np.cos((np.arange(2000) + 0.5) * np.pi / 2000)
    coef = np.polynomial.chebyshev.chebfit(xs, np.sqrt(xs + KAPPA), DEG)
    tr1 = D * (1.0 - LC) / LH

    nucs = np.zeros(C, np.float64)
    nuc_all = 0.0
    for k in range(8):
        ip = res.results[k]["out_ip"].astype(np.float64).sum(axis=0)
        for j in range(9):
            t1j = t1_all if j == 8 else t1[8 * k + j]
            if not np.isfinite(t1j) or t1j <= 1e-20:
                nuc = 0.0
            else:
                # device reports <A,A>, <P,P>, <P,A> with P = A^2;
                # T2 = 2P - I is folded in here:
                #   tr(T2) = 2<A,A> - D
                #   tr(T3) = 2<T2,A> - tr1 = 4<P,A> - 3*tr1
                #   tr(T4) = 2<T2,T2> - D = 8<P,P> - 8<A,A> + D
                ips = ip[j * IPC : (j + 1) * IPC]
                tr = np.array([D, tr1, 2 * ips[0] - D, 4 * ips[2] - 3 * tr1,
                               8 * ips[1] - 8 * ips[0] + D])
                nuc = float((coef * tr).sum() * np.sqrt(LH * t1j / D))
            if j < 8:
                nucs[8 * k + j] = nuc
            elif k == 0:
                nuc_all = nuc
    obj_c = np.maximum(nucs, DELTA).sum()
    out = (obj_c - lam_f * nuc_all) / N * lam_f
    return np.asarray(out, dtype=np.float32)


# revision 32
# speedup vs baseline: 1.0530x; 1.0530x over previous
"""Trainium2 kernel for the CLML loss function.

Math: nuclear_norm(diag(m_c) F) = tr(sqrt(G_c)) with G_c = F^T diag(m_c) F a
256x256 PSD Gram matrix.  tr(sqrt(.)) is evaluated with a matmul-only
Chebyshev trace method (degree 4):

  A  = G*s - kappa*I          (affine map of the spectrum into [-1, 1])
  T2 = 2*A*A - I
  tr(T2) = 2<A,A> - 256,  tr(T4) = 2<T2,T2> - 256,  tr(T3) = 2<T2,A> - tr(T1)

The host combines the traces with Chebyshev coefficients of sqrt(x + kappa).
tr(G_c) (hence the scale s) is computed host-side from fp32 row norms, so the
device only produces the three inner products per matrix.

Sharding: each core handles 8 classes as 4 pairs.  Pair 0's rows are sorted
into segments (11, 10, 01, 00) covering ALL N rows, so the full-matrix Gram
G_all = S11+S10+S01+S00 falls out for free.  Pairs 1-3 use the complement
trick: only segments (00, 10, 01) are contracted (~64% of rows) and
G_c0 = G_all - S00 - S01,  G_c1 = G_all - S00 - S10.

Features are fp8 e3m4 (4 mantissa bits; inputs are ~N(0,1)); the Chebyshev
recurrence runs in bf16.  Element-wise work is spread over DVE (assembly,
cross inner products), ACT (square inner products) and Pool (PSUM drains).
"""

import numpy as np
import ml_dtypes
from contextlib import ExitStack

import concourse.bass as bass
import concourse.mybir as mybir
import concourse.tile as tile
from concourse import bacc
from concourse.bass_utils import run_bass_kernel_spmd

# ---- problem constants (hardcoded; harness provides identical shapes) ----
N, C, D = 8192, 64, 256
P = 128
TAU = 0.7
MARGIN = 1.0
DELTA = 1.0

# Chebyshev spectral interval, relative to mean eigenvalue mu = tr(G)/D.
ALPHA, BETA = 0.45, 1.9
LC = (BETA + ALPHA) / 2.0
LH = (BETA - ALPHA) / 2.0
KAPPA = LC / LH
DEG = 4
IPC = 3

BF16 = mybir.dt.bfloat16
F32 = mybir.dt.float32
DT_FEAT = mybir.dt.float8e3
NP_FEAT = ml_dtypes.float8_e3m4
NP_BF16 = ml_dtypes.bfloat16

TRACE = False
LAST_RESULT = None

_PROGRAM_CACHE = {}


def _build_program(b0, a0, c0, z0, zc, cc, ac):
    """b0,a0,c0,z0: chunk counts of pair0's (11, 10, 01, 00) segments;
    zc,cc,ac: chunk counts of the complement pairs' (00, 01, 10) segments.
    Shared by all pairs and cores (zero-padded on host)."""
    CP0 = b0 + a0 + c0 + z0
    CPQ = zc + cc + ac
    CPT = CP0 + 3 * CPQ
    nc = bacc.Bacc(
        "TRN2",
        target_bir_lowering=False,
        debug=False,
        enable_asserts=False,
        num_devices=8,
    )
    fsort = nc.dram_tensor("fsort", [P, CPT * D], DT_FEAT, kind="ExternalInput").ap()
    cf32 = nc.dram_tensor("cf32", [P, 400], F32, kind="ExternalInput").ap()
    cbf16 = nc.dram_tensor("cbf16", [P, 512], BF16, kind="ExternalInput").ap()
    out_ip = nc.dram_tensor("out_ip", [P, 9 * IPC], F32, kind="ExternalOutput").ap()

    alu = mybir.AluOpType
    aft = mybir.ActivationFunctionType

    with tile.TileContext(nc) as tc, ExitStack() as ctx:
        f0pool = ctx.enter_context(tc.tile_pool(name="f0", bufs=1))
        fqpool = ctx.enter_context(tc.tile_pool(name="fq", bufs=3))
        cpool = ctx.enter_context(tc.tile_pool(name="c", bufs=1))
        gpool = ctx.enter_context(tc.tile_pool(name="gall", bufs=1))
        wpool = ctx.enter_context(tc.tile_pool(name="w", bufs=8))
        apool = ctx.enter_context(tc.tile_pool(name="amat", bufs=9))
        scrpool = ctx.enter_context(tc.tile_pool(name="scr", bufs=4))
        opool = ctx.enter_context(tc.tile_pool(name="outs", bufs=1))
        p0sum = ctx.enter_context(tc.tile_pool(name="p0", bufs=1, space="PSUM"))
        pqsum = ctx.enter_context(tc.tile_pool(name="pq", bufs=2, space="PSUM"))
        trsum = ctx.enter_context(tc.tile_pool(name="tr", bufs=1, space="PSUM"))

        # ---- input tiles + DMA (partition-major contiguous) ----
        fs0 = f0pool.tile([P, CP0, D], DT_FEAT, tag="f0")
        fsq = [fqpool.tile([P, CPQ, D], DT_FEAT, tag="fq", name=f"fq{q}")
               for q in range(3)]

        def dma_chunks(dst, base, cnt, nsplit, head=None):
            splits = [cnt * i // nsplit for i in range(nsplit + 1)]
            if head is not None:
                splits = [0] + [s for s in splits if s > head]
                splits.insert(1, head)
            for r0, r1 in zip(splits, splits[1:]):
                nc.sync.dma_start(
                    dst[:, r0:r1], fsort[:, (base + r0) * D : (base + r1) * D]
                )

        dma_chunks(fs0, 0, CP0, 9, head=2)
        cfp = cpool.tile([P, 400], F32, tag="cf")
        nc.sync.dma_start(cfp[:], cf32)
        cbt = cpool.tile([P, 512], BF16, tag="cb")
        nc.sync.dma_start(cbt[:], cbf16)
        for q in range(3):
            dma_chunks(fsq[q], CP0 + q * CPQ, CPQ, 6)

        kI = cfp[:, 0:384]        # kappa at [p, p] (top) and [p, 256+p] (br)
        svec = cfp[:, 384:400]    # per-class scale s_j at col j (j=0..8)
        T0 = cbt[:, 0:512]        # identity in [128, 512] two-row-block layout

        ip_sb = opool.tile([P, 9 * IPC], F32, tag="ip")

        gall = gpool.tile([P, 384], F32, tag="g")

        def asm_stt(j, src):
            """A_j = s_j * src - kappa*I; src is f32 [P, 384] (top+br)."""
            s = svec[:, j : j + 1]
            A = apool.tile([P, 512], BF16, tag="a", name=f"A{j}")
            nc.vector.scalar_tensor_tensor(
                A[:, 0:256], src[:, 0:256], s, kI[:, 0:256], alu.mult, alu.subtract
            )
            nc.vector.scalar_tensor_tensor(
                A[:, 384:512], src[:, 256:384], s, kI[:, 256:384],
                alu.mult, alu.subtract,
            )
            return A

        def asm_tr(A):
            # A10 = A01^T into [256:384] so A[:, 256:512] is the bottom rows
            ptr = trsum.tile([P, 128], BF16, tag="t")
            nc.tensor.transpose(ptr[:], A[:, 128:256], T0[:, 0:128])
            nc.vector.tensor_copy(A[:, 256:384], ptr[:])

        def asm_A(j, src):
            A = asm_stt(j, src)
            asm_tr(A)
            return A

        def gram_pair0(fillers=()):
            pg = p0sum.tile([P, 1536], F32, tag="g0")
            tops = [pg[:, i * 256 : (i + 1) * 256] for i in range(4)]
            brs = [pg[:, 1024 + i * 128 : 1024 + (i + 1) * 128] for i in range(4)]
            bounds = [0, b0, b0 + a0, b0 + a0 + c0, CP0]
            fill = sorted(fillers, key=lambda x: x[0], reverse=True)
            for i in range(4):
                lo, hi = bounds[i], bounds[i + 1]
                for n in range(lo, hi):
                    Fn = fs0[:, n]
                    nc.tensor.matmul(
                        tops[i], Fn[:, 0:128], Fn, start=(n == lo), stop=(n == hi - 1)
                    )
                    nc.tensor.matmul(
                        brs[i], Fn[:, 128:256], Fn[:, 128:256],
                        start=(n == lo), stop=(n == hi - 1),
                    )
                    while fill and fill[-1][0] <= n:
                        fill.pop()[1](tops, brs)
            while fill:
                fill.pop()[1](tops, brs)
            return pg, tops, brs

        # pair0 drain pieces, each placed at its earliest legal point in the
        # gram stream.  GPSIMD has no PSUM access and engines take at most
        # one PSUM operand per op: ACT copies S11/S01 out, DVE adds with one
        # PSUM side.
        st = {}

        def d0_copy11(tops, brs):
            c11 = wpool.tile([P, 384], F32, tag="w", name="c11")
            nc.scalar.copy(c11[:, 0:256], tops[0])
            nc.scalar.copy(c11[:, 256:384], brs[0])
            st["c11"] = c11

        def d0_classA(tops, brs):
            c11 = st["c11"]
            t01 = wpool.tile([P, 384], F32, tag="w", name="t01")
            nc.vector.tensor_add(t01[:, 0:256], c11[:, 0:256], tops[1])
            nc.vector.tensor_add(t01[:, 256:384], c11[:, 256:384], brs[1])
            st["t01"] = t01
            st["A0"] = asm_stt(0, t01)

        def d0_classB(tops, brs):
            c11 = st["c11"]
            c01 = wpool.tile([P, 384], F32, tag="w", name="c01")
            nc.scalar.copy(c01[:, 0:256], tops[2])
            nc.scalar.copy(c01[:, 256:384], brs[2])
            t02 = wpool.tile([P, 384], F32, tag="w", name="t02")
            nc.vector.tensor_add(t02[:], c11[:], c01[:])
            st["c01"] = c01
            st["A1"] = asm_stt(1, t02)

        def d0_solo(tops, brs):
            c01 = st["c01"]
            t23 = wpool.tile([P, 384], F32, tag="w", name="t23")
            nc.vector.tensor_add(t23[:, 0:256], c01[:, 0:256], tops[3])
            nc.vector.tensor_add(t23[:, 256:384], c01[:, 256:384], brs[3])
            nc.vector.tensor_add(gall[:], st["t01"][:], t23[:])
            st["A8"] = asm_stt(8, gall)

        def gram_pairq(q, fillers=()):
            # segments: 0 -> 00, 1 -> 01, 2 -> 10.  The 00 segment's br
            # matmuls accumulate into BOTH classes' br accumulators directly
            # (br0 = S00b+S01b, br1 = S00b+S10b) to fit the pair in 2 banks.
            # `fillers`: (after_chunk, fn) callbacks emitted mid-stream so
            # trailing cheb matmuls are spaced out in the PE queue.
            fst = fsq[q - 1]
            pg = pqsum.tile([P, 1024], F32, tag="gq", name=f"gq{q}")
            tops = [pg[:, i * 256 : (i + 1) * 256] for i in range(3)]
            br0 = pg[:, 768:896]
            br1 = pg[:, 896:1024]
            bounds = [0, zc, zc + cc, CPQ]
            fill = sorted(fillers, key=lambda x: x[0], reverse=True)
            for i in range(3):
                lo, hi = bounds[i], bounds[i + 1]
                for n in range(lo, hi):
                    Fn = fst[:, n]
                    nc.tensor.matmul(
                        tops[i], Fn[:, 0:128], Fn, start=(n == lo), stop=(n == hi - 1)
                    )
                    Fb = Fn[:, 128:256]
                    if i == 0:
                        nc.tensor.matmul(br0, Fb, Fb, start=(n == lo), stop=False)
                        nc.tensor.matmul(br1, Fb, Fb, start=(n == lo), stop=False)
                    elif i == 1:
                        nc.tensor.matmul(br0, Fb, Fb, start=False, stop=(n == hi - 1))
                    else:
                        nc.tensor.matmul(br1, Fb, Fb, start=False, stop=(n == hi - 1))
                    while fill and fill[-1][0] <= n:
                        fill.pop()[1](tops, (br0, br1))
            while fill:
                fill.pop()[1](tops, (br0, br1))
            return pg, tops, (br0, br1)

        def drain_classA(q, tops, brs):
            """class 2q: G = gall - (S00 + S01); ready once segs 00,01 stop."""
            br0, br1 = brs
            c00 = wpool.tile([P, 256], F32, tag="w", name=f"c00_{q}")
            u0 = wpool.tile([P, 256], F32, tag="w", name=f"u0_{q}")
            w0 = wpool.tile([P, 384], F32, tag="w", name=f"w0_{q}")
            nc.scalar.copy(c00[:], tops[0])
            nc.vector.tensor_add(u0[:], c00[:], tops[1])
            nc.vector.tensor_sub(w0[:, 0:256], gall[:, 0:256], u0[:])
            nc.vector.tensor_sub(w0[:, 256:384], gall[:, 256:384], br0)
            return asm_stt(2 * q, w0), c00

        def drain_classB(q, tops, brs, c00):
            """class 2q+1: G = gall - (S00 + S10)."""
            br0, br1 = brs
            u1 = wpool.tile([P, 256], F32, tag="w", name=f"u1_{q}")
            w1 = wpool.tile([P, 384], F32, tag="w", name=f"w1_{q}")
            nc.vector.tensor_add(u1[:], c00[:], tops[2])
            nc.vector.tensor_sub(w1[:, 0:256], gall[:, 0:256], u1[:])
            nc.vector.tensor_sub(w1[:, 256:384], gall[:, 256:384], br1)
            return asm_stt(2 * q + 1, w1)

        def drain_pairq(q, pg, tops, brs):
            A0, c00 = drain_classA(q, tops, brs)
            asm_tr(A0)
            A1 = drain_classB(q, tops, brs, c00)
            asm_tr(A1)
            return [(A0, 2 * q), (A1, 2 * q + 1)]

        def cheb(A, j):
            """Inner products <A,A>, <P,P>, <P,A> with P = A*A left in PSUM;
            the host folds T2 = 2P - I into the trace formulas."""
            base = j * IPC
            scr = scrpool.tile([P, 512], F32, tag="scr")
            nc.scalar.activation(
                scr[:], A[:], aft.Square, accum_out=ip_sb[:, base : base + 1]
            )
            # rotate pair0's psum buffer (drained by then); plain pool-tile
            # rotation gives a clean WAR edge vs the previous cheb's reads
            ppt = p0sum.tile([P, 1536], F32, tag="g0", name=f"pp{j}")
            pp = ppt[:, 0:512]
            for mb in (0, 1):
                pm = pp[:, mb * 256 : mb * 256 + 256]
                nc.tensor.matmul(
                    pm, A[:, mb * 128 : mb * 128 + 128], A[:, 0:256],
                    start=True, stop=False,
                )
                nc.tensor.matmul(
                    pm, A[:, 256 + mb * 128 : 256 + mb * 128 + 128], A[:, 256:512],
                    start=False, stop=True,
                )
            scr2 = scrpool.tile([P, 512], F32, tag="scr")
            nc.scalar.activation(
                scr2[:], pp, aft.Square, accum_out=ip_sb[:, base + 1 : base + 2]
            )
            scr3 = scrpool.tile([P, 512], F32, tag="scr")
            nc.vector.scalar_tensor_tensor(
                scr3[:], pp, 1.0, A[:], alu.mult, alu.mult,
                accum_out=ip_sb[:, base + 2 : base + 3],
            )

        # ---- schedule: pair grams lead the PE queue; drain pieces are
        # placed at their earliest legal point (dependencies coarsen to
        # program order, so placement IS the sync point); chebs are spaced
        # through the next pair's gram chunks so the p0-psum rotation WAR
        # is satisfied before the PE reaches each one ----
        f = lambda fn: (lambda tops, brs: fn())

        def mk_drainA(q):
            def go(tops, brs):
                st[f"A{2 * q}"], st[f"c{q}"] = drain_classA(q, tops, brs)
            return go

        pg0 = gram_pair0(fillers=[
            (b0, d0_copy11),
            (b0 + a0, d0_classA),
            (min(b0 + a0 + 4, CP0 - 2), f(lambda: asm_tr(st["A0"]))),
            (b0 + a0 + c0, d0_classB),
            (min(b0 + a0 + c0 + 3, CP0 - 1), f(lambda: asm_tr(st["A1"]))),
        ])
        d0_solo(pg0[1], pg0[2])
        pq1 = gram_pairq(1, fillers=[
            (6, f(lambda: cheb(st["A0"], 0))),
            (8, f(lambda: asm_tr(st["A8"]))),
            (13, f(lambda: cheb(st["A1"], 1))),
            (20, f(lambda: cheb(st["A8"], 8))),
            (zc + cc - 1, mk_drainA(1)),
            (min(zc + cc + 6, CPQ - 5), f(lambda: asm_tr(st["A2"]))),
            (min(zc + cc + 11, CPQ - 1), f(lambda: cheb(st["A2"], 2))),
        ])
        st["A3"] = drain_classB(1, pq1[1], pq1[2], st["c1"])
        pq2 = gram_pairq(2, fillers=[
            (2, f(lambda: asm_tr(st["A3"]))),
            (6, f(lambda: cheb(st["A3"], 3))),
            (zc + cc - 1, mk_drainA(2)),
            (min(zc + cc + 6, CPQ - 5), f(lambda: asm_tr(st["A4"]))),
            (min(zc + cc + 11, CPQ - 1), f(lambda: cheb(st["A4"], 4))),
        ])
        st["A5"] = drain_classB(2, pq2[1], pq2[2], st["c2"])
        pq3 = gram_pairq(3, fillers=[
            (2, f(lambda: asm_tr(st["A5"]))),
            (6, f(lambda: cheb(st["A5"], 5))),
            (zc + cc - 1, mk_drainA(3)),
            (min(zc + cc + 6, CPQ - 5), f(lambda: asm_tr(st["A6"]))),
            (min(zc + cc + 11, CPQ - 1), f(lambda: cheb(st["A6"], 6))),
        ])
        A7 = drain_classB(3, pq3[1], pq3[2], st["c3"])
        asm_tr(A7)
        cheb(A7, 7)

        nc.sync.dma_start(out_ip, ip_sb[:])

    nc.compile()
    return nc


def _get_program(key):
    if key not in _PROGRAM_CACHE:
        _PROGRAM_CACHE[key] = _build_program(*key)
    return _PROGRAM_CACHE[key]


def _host_consts():
    kI = np.zeros((P, 384), np.float32)
    for p in range(P):
        kI[p, p] = KAPPA
        kI[p, 256 + p] = KAPPA
    T0 = np.zeros((P, 512), np.float32)
    for p in range(P):
        T0[p, p] = 1.0
        T0[p, 384 + p] = 1.0
    return kI, T0.astype(NP_BF16)


def kernel(logits, targets, feature, lam, epoch):
    global LAST_RESULT
    logits = np.asarray(logits, dtype=np.float32)
    targets_b = np.asarray(targets) == 1
    feature = np.asarray(feature, dtype=np.float32)
    lam_f = float(np.asarray(lam))
    relabel = int(np.asarray(epoch)) >= 1

    # masks (same fp32 semantics as the reference)
    if relabel:
        shifted = (logits - targets_b.astype(np.float32)).astype(np.float32)
        thresh = np.float32(np.log(TAU / (1.0 - TAU)))
        mask = targets_b | (shifted > thresh)
    else:
        mask = targets_b.copy()

    feat8 = np.ascontiguousarray(feature.astype(NP_FEAT))
    kI, T0 = _host_consts()

    # host-side traces: tr(G_c) = sum of masked row norms (fp64-exact)
    rn = (feature.astype(np.float64) ** 2).sum(axis=1)
    t1 = rn @ mask  # [C]
    t1_all = float(rn.sum())

    # ---- per-core, per-pair sorted row layout ----
    # pair 0: segments (11, 10, 01, 00); pairs 1-3: complement (00, 10, 01)
    idx = {}
    for k in range(8):
        m0 = mask[:, 8 * k]
        m1 = mask[:, 8 * k + 1]
        idx[(k, 0)] = [
            np.where(m0 & m1)[0], np.where(m0 & ~m1)[0],
            np.where(~m0 & m1)[0], np.where(~m0 & ~m1)[0],
        ]
        for q in range(1, 4):
            m0 = mask[:, 8 * k + 2 * q]
            m1 = mask[:, 8 * k + 2 * q + 1]
            idx[(k, q)] = [
                np.where(~m0 & ~m1)[0], np.where(~m0 & m1)[0],
                np.where(m0 & ~m1)[0],
            ]

    def nch(x):
        return max((len(x) + P - 1) // P, 1)

    cnt0 = [max(nch(idx[(k, 0)][i]) for k in range(8)) for i in range(4)]
    cntq = [max(nch(idx[(k, q)][i]) for k in range(8) for q in range(1, 4))
            for i in range(3)]
    key = tuple(cnt0) + tuple(cntq)
    CP0 = sum(cnt0)
    CPQ = sum(cntq)
    CPT = CP0 + 3 * CPQ

    in_maps = []
    for k in range(8):
        fsort = np.zeros((CPT * P, D), NP_FEAT)
        off = 0
        for q in range(4):
            cnts = cnt0 if q == 0 else cntq
            for rows, segc in zip(idx[(k, q)], cnts):
                fsort[off : off + len(rows)] = feat8[rows]
                off += segc * P
        fsort_pm = np.ascontiguousarray(
            fsort.reshape(CPT, P, D).transpose(1, 0, 2).reshape(P, CPT * D)
        )
        svec = np.zeros((P, 16), np.float32)
        for j in range(8):
            svec[:, j] = D / (LH * max(t1[8 * k + j], 1e-30))
        svec[:, 8] = D / (LH * max(t1_all, 1e-30))
        cf32 = np.ascontiguousarray(
            np.concatenate([kI, svec], axis=1).astype(np.float32)
        )
        in_maps.append({"fsort": fsort_pm, "cf32": cf32, "cbf16": T0})

    nc = _get_program(key)
    res = run_bass_kernel_spmd(nc, in_maps, core_ids=list(range(8)), trace=TRACE)
    LAST_RESULT = res

    # ---- host combination ----
    xs = np.cos((np.arange(2000) + 0.5) * np.pi / 2000)
    coef = np.polynomial.chebyshev.chebfit(xs, np.sqrt(xs + KAPPA), DEG)
    tr1 = D * (1.0 - LC) / LH

    nucs = np.zeros(C, np.float64)
    nuc_all = 0.0
    for k in range(8):
        ip = res.results[k]["out_ip"].astype(np.float64).sum(axis=0)
        for j in range(9):
            t1j = t1_all if j == 8 else t1[8 * k + j]
            if not np.isfinite(t1j) or t1j <= 1e-20:
                nuc = 0.0
            else:
                # device reports <A,A>, <P,P>, <P,A> with P = A^2;
                # T2 = 2P - I is folded in here:
                #   tr(T2) = 2<A,A> - D
                #   tr(T3) = 2<T2,A> - tr1 = 4<P,A> - 3*tr1
                #   tr(T4) = 2<T2,T2> - D = 8<P,P> - 8<A,A> + D
                ips = ip[j * IPC : (j + 1) * IPC]
                tr = np.array([D, tr1, 2 * ips[0] - D, 4 * ips[2] - 3 * tr1,
                               8 * ips[1] - 8 * ips[0] + D])
                nuc = float((coef * tr).sum() * np.sqrt(LH * t1j / D))
            if j < 8:
                nucs[8 * k + j] = nuc
            elif k == 0:
                nuc_all = nuc
    obj_c = np.maximum(nucs, DELTA).sum()
    out = (obj_c - lam_f * nuc_all) / N * lam_f
    return np.asarray(out, dtype=np.float32)
